# revision 18
# baseline (speedup 1.0000x reference)
"""Trainium2 Bass kernel for the quantized ResNet Bottleneck block (v3).

Strategy
--------
Data parallel over batch: 64 images -> 8 cores x 8 images. Each core runs an
identical Bass program; weights are replicated.

All convs are integer-valued matmuls accumulated in fp32 PSUM (exact:
quantized codes are integers; code offsets pass through each conv as
per-output-channel constants folded into the next bias on the host).

v3 numeric scheme (simpler + more exact than v2):
  * x is fake-quantized ON THE HOST: codes c0 = clip(rne(254*x), -127, 127)
    shipped PLAIN (no offset) as bf16 (ints <= 127 are exact). This removes
    the on-device x-quant DVE passes entirely and the shortcut conv needs
    no column-sum correction.
  * conv1/conv2 epilogues are ONE custom DVE op each (QEPI):
    out = clip(rne(a*psum + beta), 0, 127) + 128, rounding via the f32
    magic-add (v + 1.5*2^23) trick -- exact RNE on the integer grid.
    Outputs (codes+128 in [1,255]) are exact in bf16. Pad value for conv2
    is 128; the +128 offsets fold into the next layer's bias via weight
    column sums.
  * conv3 + stride-2 shortcut accumulate into one PSUM tile (shortcut
    weights pre-scaled by css/c3s on host); epilogue is ONE custom DVE op
    (FEPI): y = min(relu(g3*psum + delta), 6), bf16 out, per-m store.

Engine placement (learned from HW traces):
  * NO elementwise work on GpSimd: Q7 tensor ops starve SBUF arbitration
    for every other engine (DVE ops degrade 350ns -> 6us while a GpSimd
    tensor_scalar runs). GpSimd only does pad memsets + tiny SWDGE DMAs.
  * DVE runs all epilogues as fused custom ops (~0.55us per [128,392]).
  * ACT only builds per-channel bias broadcast tiles once at startup.
  * Weights ride the ACT HWDGE queue, x codes + y stores the SP queue, so
    the weight stream does not delay the x stream.
  * DVE APs kept strictly 2D-contiguous where possible (3-dim views drop
    DVE to 1 elem/cycle on HW).

Schedule: pair 0 streams per (k,i) half-chunks so conv1 starts ~7us in;
software pipeline: conv2/conv3 of pair p-1 run under conv1 of pair p. A
short run of dummy matmuls on zeroed tiles warms the PE p-state ramp while
the first x DMA flies.
"""

import sys
from contextlib import ExitStack

import numpy as np

sys.path.insert(0, "/opt/trn_rl_repo")

import ml_dtypes  # noqa: E402

import concourse.bacc as bacc  # noqa: E402
import concourse.dve_ops as dve_ops  # noqa: E402
import concourse.tile as tile  # noqa: E402
from concourse import mybir  # noqa: E402
from concourse.bass_utils import run_bass_kernel_spmd  # noqa: E402
from concourse.dve_spec import (  # noqa: E402
    C0 as DC0,
    C1 as DC1,
    C2 as DC2,
    One as DOne,
    Spec,
    Src0 as DSrc0,
    Src1 as DSrc1,
    _has_src1,
    lower as dve_lower,
    maxx,
    minn,
    relu as drelu,
)
from concourse.dve_uop import DveOpSpec  # noqa: E402
from concourse.dve_table_gen import dve_ver_for  # noqa: E402
from concourse.dve_ops import DveOp  # noqa: E402

F32 = mybir.dt.float32
F16 = mybir.dt.float16
BF16 = mybir.dt.bfloat16
ALU = mybir.AluOpType
AFT = mybir.ActivationFunctionType
BF16NP = ml_dtypes.bfloat16

C_MAGIC = float(np.float32(12582912.0))  # 1.5 * 2**23

N_CORES = 8
B_LOC = 8  # images per core
PAIRS = B_LOC // 2


def _register_dve_op(name, spec, subdim=False):
    """Register a custom DVE op at runtime (table is generated per-NEFF)."""
    for o in dve_ops.OPS:
        if o.name == name:
            return o
    row = dve_ops._CUSTOM_DVE_ROW_BASE + len(dve_ops.OPS)
    assert row < 0x20
    shas = {}
    for ver in ("v3", "v4"):
        tmp = DveOpSpec(
            name=name, opcode=row, uops=dve_lower(spec, ver=ver),
            rd1_en=_has_src1(spec),
        )
        shas[ver] = tmp.sha(ver)
    op = DveOp(name, spec, subdim=subdim, uops_sha=shas)
    dve_ops.OPS.append(op)
    dve_ops._SUB_OPCODE_FOR_NAME[name] = row
    dve_ops.CUSTOM_DVE_SPECS[name] = spec
    return op


def _b(in0, in1):
    """Sim helper: in1 streams elementwise on HW; align shapes for numpy."""
    if isinstance(in1, np.ndarray) and in1.size == in0.size:
        return in1.reshape(in0.shape)
    return in1


# quantize epilogue: v = in0*s0 + in1 ; out = clip(rne(v),0,127) + 128
# rne via magic-add: u = v + C rounds to the integer grid (f32 ulp==1 in
# [2^23, 2^24)); clip in the shifted domain [C, C+127]; subtract C-128.
QEPI = _register_dve_op(
    "BNECK_QEPI_ANT",
    Spec(
        body=(minn(maxx((DSrc0 * DC0 + DSrc1) + DC1, DC1), DC1 + DC2) - DC1)
        + (DC2 + DOne),
        reference=lambda in0, in1, s0, s1, imm2: np.minimum(
            np.maximum(np.round(in0 * s0 + _b(in0, in1)), 0.0), imm2
        )
        + (imm2 + 1.0),
    ),
)


# final epilogue: out = min(relu(in0*s0 + in1), s1)
FEPI = _register_dve_op(
    "BNECK_FEPI_ANT",
    Spec(
        body=minn(drelu(DSrc0 * DC0 + DSrc1), DC1),
        reference=lambda in0, in1, s0, s1, imm2: np.minimum(
            np.maximum(in0 * s0 + _b(in0, in1), 0.0), s1
        ),
    ),
)


def _build_nc(pairs=PAIRS):
    nc = bacc.Bacc("TRN2", target_bir_lowering=False, debug=False)

    # x codes, plain int values in [-127,127], layout [pair, q, k, (i h)]
    x_d = nc.dram_tensor("x", [pairs, 128, 4, 1568], BF16, kind="ExternalInput")
    w1_d = nc.dram_tensor("w1l", [4, 128, 256], BF16, kind="ExternalInput")
    w2_d = nc.dram_tensor("w2l", [9, 2, 128, 256], BF16, kind="ExternalInput")
    w3_d = nc.dram_tensor("w3l", [2, 128, 1024], BF16, kind="ExternalInput")
    ws_d = nc.dram_tensor("wsl", [4, 128, 1024], BF16, kind="ExternalInput")
    b1_d = nc.dram_tensor("beta1", [128, 2], F32, kind="ExternalInput")
    b2_d = nc.dram_tensor("beta2", [128, 2], F32, kind="ExternalInput")
    dl_d = nc.dram_tensor("delta", [128, 8], F32, kind="ExternalInput")
    # packed [pair, m-pair, q, (m2 i h)] to keep the out-DMA 2D; host unpacks
    y_d = nc.dram_tensor("y", [pairs, 4, 128, 784], BF16, kind="ExternalOutput")

    a1, a2, g3 = _SCALES

    with tile.TileContext(nc) as tc, ExitStack() as ctx:
        wp = ctx.enter_context(tc.tile_pool(name="w", bufs=1))
        xqp = ctx.enter_context(tc.tile_pool(name="xq", bufs=3))
        p2p = ctx.enter_context(tc.tile_pool(name="p2", bufs=2))
        t3p = ctx.enter_context(tc.tile_pool(name="t3", bufs=2))
        yop = ctx.enter_context(tc.tile_pool(name="yo", bufs=6))
        pc1 = ctx.enter_context(tc.tile_pool(name="pc1", bufs=3, space="PSUM"))
        pc2 = ctx.enter_context(tc.tile_pool(name="pc2", bufs=2, space="PSUM"))
        pc3 = ctx.enter_context(tc.tile_pool(name="pc3", bufs=3, space="PSUM"))

        # ---- weights + biases ----
        # startup critical path: w1's k0 slice is the very first ACT-queue
        # DMA while pair-0's first x chunk is the first SP DMA — the two
        # sequencers issue in parallel (each issue costs ~0.6-1us of
        # sequencer time, so queue assignment matters more than transfer
        # time). Bulk weights follow on ACT, x pairs + nothing else on SP,
        # y stores on ACT, tiny biases on the GpSimd SWDGE queue.
        w1t = wp.tile([128, 4, 256], BF16, tag="w1t")
        nc.scalar.dma_start(w1t[:, 0], w1_d[0])

        beta1 = wp.tile([128, 2], F32, tag="beta1")
        nc.gpsimd.dma_start(beta1[:], b1_d[:])
        beta2 = wp.tile([128, 2], F32, tag="beta2")
        nc.gpsimd.dma_start(beta2[:], b2_d[:])
        delta = wp.tile([128, 8], F32, tag="delta")
        nc.gpsimd.dma_start(delta[:], dl_d[:])

        # zero tile: ACT broadcast source + PE warmup operands
        rz = wp.tile([128, 392], BF16, tag="rz")
        nc.vector.memset(rz[:], 0.0)
        # warm the PE p-state ramp with dummy matmuls on the zero tile
        pz = pc1.tile([128, 392], F32, tag="ps1", name="pz")
        for _ in range(8):
            nc.tensor.matmul(pz[:], rz[:, :128], rz[:], start=True, stop=True)
        # preload the ACT function table while the first DMAs fly
        warm_act = wp.tile([128, 1], F32, tag="warm_act")
        nc.scalar.activation(warm_act[:], rz[:, :1], AFT.Relu, bias=0.0, scale=1.0)

        st = {}  # per-pair tiles: xq, p2, t3

        def emit_xload(p, fine=False):
            """x codes for a pair land straight in matmul layout. Pair 0
            streams per (k,i) half-chunk so conv1 can start early."""
            xq = xqp.tile([128, 4, 1568], BF16, tag="xq", name=f"xq_{p}")
            st[p] = {"xq": xq}
            if fine == 2:
                # pair 0: k0-i0 alone (gates the first real matmul), then
                # i0 k1-3, then all of i1 — few issues, early first chunk
                nc.sync.dma_start(xq[:, 0, 0:784], x_d[p][:, 0, 0:784])
                nc.sync.dma_start(xq[:, 1:4, 0:784], x_d[p][:, 1:4, 0:784])
                nc.sync.dma_start(
                    xq[:, :, 784:1568], x_d[p][:, :, 784:1568]
                )
            elif fine:
                # pair 1: one DMA per image half
                for i in (0, 1):
                    nc.sync.dma_start(
                        xq[:, :, i * 784 : i * 784 + 784],
                        x_d[p][:, :, i * 784 : i * 784 + 784],
                    )
            else:
                nc.sync.dma_start(xq[:], x_d[p])

        def emit_conv1(p):
            xq = st[p]["xq"]
            p2 = []
            for m in range(2):
                # pad value 128 == quantized zero in the +128-shifted domain.
                # Only the cells conv2 actually reads need padding: row 0
                # and column 1 of each 29x32 image plane.
                t = p2p.tile([128, 1856], BF16, tag=f"p2_{m}")
                pv = t.rearrange("q (i r c) -> q i r c", i=2, r=29, c=32)
                nc.gpsimd.memset(pv[:, :, 0, :], 128.0)
                nc.gpsimd.memset(pv[:, :, 1:29, 1], 128.0)
                p2.append(t)
            for i in (0, 1):
                ps = {}
                # k-outer across the four (m, hf) quads; three PSUM tiles
                # from pc1, the fourth borrows a pc3 bank
                quads = [(0, 0), (0, 1), (1, 0), (1, 1)]
                for k in range(4):
                    for m, hf in quads:
                        if k == 0:
                            pool = pc3 if (m, hf) == (1, 1) else pc1
                            ps[(m, hf)] = pool.tile(
                                [128, 392], F32,
                                name=f"ps1_{m}_{hf}",
                                tag="ps3" if (m, hf) == (1, 1) else "ps1",
                            )
                        nc.tensor.matmul(
                            ps[(m, hf)][:],
                            w1t[:, k, m * 128 : (m + 1) * 128],
                            xq[:, k, i * 784 + hf * 392 : i * 784 + hf * 392 + 392],
                            start=(k == 0),
                            stop=(k == 3),
                            skip_group_check=True,
                        )
                for m in (0, 1):
                    pv = p2[m].rearrange("q (i r c) -> q i r c", i=2, r=29, c=32)
                    for hf in (0, 1):
                        # fused epilogue: codes+128 via exact magic-add RNE
                        nc.vector._custom_dve(
                            QEPI,
                            out=pv[:, i, 1 + 14 * hf : 15 + 14 * hf, 2:30],
                            in0=ps[(m, hf)][:],
                            in1=bc1[m][:],
                            s0=a1, s1=C_MAGIC, imm2=127.0,
                        )
            st[p]["p2"] = p2

        def emit_conv2(p):
            p2 = st[p]["p2"]
            t3 = []
            for m in range(2):
                ps2 = pc2.tile([128, 392], F32)
                first = True
                for k in range(2):
                    pv = p2[k].rearrange("q (i r c) -> q i r c", i=2, r=29, c=32)
                    for tp in range(9):
                        ky, kx = divmod(tp, 3)
                        nc.tensor.matmul(
                            ps2[:],
                            w2t[:, tp, k, m * 128 : (m + 1) * 128],
                            pv[:, :, ky : min(ky + 28, 29) : 2, 1 + kx : 29 + kx : 2],
                            start=first,
                            stop=(k == 1 and tp == 8),
                        )
                        first = False
                t3m = t3p.tile([128, 392], BF16, tag=f"t3_{m}")
                nc.vector._custom_dve(
                    QEPI, out=t3m[:], in0=ps2[:], in1=bc2[m][:],
                    s0=a2, s1=C_MAGIC, imm2=127.0,
                )
                t3.append(t3m)
            st[p]["t3"] = t3

        def emit_conv3(p):
            xq, t3 = st[p]["xq"], st[p]["t3"]
            for m in range(8):
                ps3 = pc3.tile([128, 392], F32)
                # shortcut first: only needs xq, giving t3's epilogue time
                for k in range(4):
                    xv = xq[:, k].rearrange("q (i r c) -> q i r c", r=28, c=28)
                    nc.tensor.matmul(
                        ps3[:],
                        wst[:, k, m * 128 : (m + 1) * 128],
                        xv[:, :, 0:28:2, 0:28:2],
                        start=(k == 0),
                        stop=False,
                        skip_group_check=True,
                    )
                for k in range(2):
                    nc.tensor.matmul(
                        ps3[:],
                        w3t[:, k, m * 128 : (m + 1) * 128],
                        t3[k][:],
                        start=False,
                        stop=(k == 1),
                        skip_group_check=True,
                    )
                # single fused DVE epilogue + per-m store keeps the chain
                # after each m's last matmul short and GpSimd-free
                yo = yop.tile([128, 392], BF16, tag="yot", name="yot")
                nc.vector._custom_dve(
                    FEPI, out=yo[:], in0=ps3[:], in1=dlf[m][:],
                    s0=g3, s1=6.0,
                )
                nc.scalar.dma_start(
                    y_d[p, m // 2][:, (m % 2) * 392 : (m % 2) * 392 + 392],
                    yo[:],
                )
            del st[p]

        # w1 k1..k3 on the ACT HWDGE queue (k0 already inbound on SP)
        nc.scalar.dma_start(
            w1t[:, 1:4], w1_d[1:4].rearrange("k q n -> q k n")
        )

        # pair 0+1 x loads first on the SP queue
        emit_xload(0, fine=2)
        emit_xload(1, fine=True)

        # conv1's bias broadcasts first (QEPI needs them ~8us in), then the
        # bulk weights on the ACT HWDGE queue (parallel to the x stream),
        # then the later-needed broadcasts. Identity(0*rz + bias_j).
        bc1, bc2, dlf = [], [], []
        for m in range(2):
            t = wp.tile([128, 392], F32, tag=f"bc1_{m}", name=f"bc1_{m}")
            nc.scalar.activation(
                t[:], rz[:], AFT.Identity, bias=beta1[:, m : m + 1], scale=0.0
            )
            bc1.append(t)
        w2t = wp.tile([128, 9, 2, 256], BF16, tag="w2t")
        nc.scalar.dma_start(w2t[:], w2_d.rearrange("t k q n -> q t k n"))
        wst = wp.tile([128, 4, 1024], BF16, tag="wst")
        nc.scalar.dma_start(wst[:], ws_d.rearrange("k q n -> q k n"))
        w3t = wp.tile([128, 2, 1024], BF16, tag="w3t")
        nc.scalar.dma_start(w3t[:], w3_d.rearrange("k q n -> q k n"))
        for m in range(2):
            t = wp.tile([128, 392], F32, tag=f"bc2_{m}", name=f"bc2_{m}")
            nc.scalar.activation(
                t[:], rz[:], AFT.Identity, bias=beta2[:, m : m + 1], scale=0.0
            )
            bc2.append(t)
        for j in range(8):
            t = wp.tile([128, 392], F32, tag=f"dlf{j}", name=f"dlf{j}")
            nc.scalar.activation(
                t[:], rz[:], AFT.Identity, bias=delta[:, j : j + 1], scale=0.0
            )
            dlf.append(t)

        emit_conv1(0)

        # software pipeline: conv2/conv3 of pair p-1 run under conv1 of pair p
        for p in range(1, pairs):
            emit_conv1(p)
            emit_conv2(p - 1)
            if p + 1 < pairs:
                emit_xload(p + 1)
            emit_conv3(p - 1)
        emit_conv2(pairs - 1)
        emit_conv3(pairs - 1)
    return nc


_SCALES = (1.0, 1.0, 1.0)


def _prep(w1, b1, w2, b2, w3, b3, wsw, bs):
    """Host-side weight quantization + constant folding (all tiny tensors)."""
    f32 = np.float32

    def qw(w):
        s = f32(np.max(np.abs(w)))
        wq = np.round(np.clip(w / s, f32(-1.0), f32(1.0)) * f32(127.0)).astype(
            np.float32
        )
        return wq, s

    def qb(b):
        return np.round(b * f32(127.0)).astype(np.float32)

    w1q, c1s = qw(w1)  # [256,512,1,1]
    w2q, c2s = qw(w2)  # [256,256,3,3]
    w3q, c3s = qw(w3)  # [1024,256,1,1]
    wsq, css = qw(wsw)  # [1024,512,1,1]
    B1, B2, B3, Bs = qb(b1), qb(b2), qb(b3), qb(bs)

    a1 = f32(2.0) * c1s / f32(127.0)
    a2 = f32(2.0) * c2s / f32(127.0)
    g3 = c3s / f32(2.0 * 16129.0)
    rho = css / c3s

    # lhsT layouts
    w1l = np.ascontiguousarray(
        w1q[:, :, 0, 0].T.reshape(4, 128, 256).astype(BF16NP)
    )
    # w2 taps: [ky,kx] -> lhsT [cin, cout] per tap
    w2l = np.ascontiguousarray(
        w2q.transpose(2, 3, 1, 0).reshape(9, 2, 128, 256).astype(BF16NP)
    )
    w3l = np.ascontiguousarray(
        w3q[:, :, 0, 0].T.reshape(2, 128, 1024).astype(BF16NP)
    )
    ws_sc = (rho * wsq[:, :, 0, 0]).astype(BF16NP)  # [1024,512] scaled bf16
    wsl = np.ascontiguousarray(ws_sc.T.reshape(4, 128, 1024))

    # column sums for the +128 code-offset corrections (fp64 exact)
    K2 = w2q.astype(np.float64).sum(axis=(1, 2, 3))  # [256]
    K3 = w3q[:, :, 0, 0].astype(np.float64).sum(axis=1)  # [1024]

    # x codes ship plain (no offset); conv1/conv2 outputs carry +128
    beta1 = (f32(4.0) * B1).astype(np.float32)
    beta2 = (f32(4.0) * B2 - a2 * f32(128.0) * K2.astype(np.float32)).astype(
        np.float32
    )
    delta0 = B3 * c3s / (f32(127.0) * c2s) + Bs / f32(127.0)
    delta = (
        delta0 - (g3.astype(np.float64) * (128.0 * K3)).astype(np.float32)
    ).astype(np.float32)

    beta1 = np.ascontiguousarray(beta1.reshape(2, 128).T)  # [128,2]
    beta2 = np.ascontiguousarray(beta2.reshape(2, 128).T)
    delta = np.ascontiguousarray(delta.reshape(8, 128).T)  # [128,8]

    return dict(
        w1l=w1l, w2l=w2l, w3l=w3l, wsl=wsl,
        beta1=beta1, beta2=beta2, delta=delta,
        a1=float(a1), a2=float(a2), g3=float(g3),
    )


def _quant_x(x):
    """Host fake-quant of x: codes = clip(rne(x/0.5 clipped * 127)) as bf16,
    laid out [core, pair, k, q, (i h)]."""
    f32 = np.float32
    c = np.round(
        np.clip(x.astype(np.float32) / f32(0.5), f32(-1.0), f32(1.0)) * f32(127.0)
    )
    # [64, 512, 28, 28] -> [cores, pairs, 2(i), 4(k), 128(q), 784(h)]
    c = c.reshape(N_CORES, PAIRS, 2, 4, 128, 784)
    # -> [cores, pairs, q, k, i, h]
    c = c.transpose(0, 1, 4, 3, 2, 5)
    return np.ascontiguousarray(
        c.reshape(N_CORES, PAIRS, 128, 4, 1568).astype(BF16NP)
    )


def kernel(x, w1, b1, w2, b2, w3, b3, ws, bs):
    global _SCALES
    xc = _quant_x(np.asarray(x, np.float32))
    pre = _prep(
        np.asarray(w1, np.float32), np.asarray(b1, np.float32),
        np.asarray(w2, np.float32), np.asarray(b2, np.float32),
        np.asarray(w3, np.float32), np.asarray(b3, np.float32),
        np.asarray(ws, np.float32), np.asarray(bs, np.float32),
    )
    _SCALES = (pre["a1"], pre["a2"], pre["g3"])
    nc = _build_nc()
    nc.compile()

    shared = {
        "w1l": pre["w1l"], "w2l": pre["w2l"], "w3l": pre["w3l"],
        "wsl": pre["wsl"], "beta1": pre["beta1"], "beta2": pre["beta2"],
        "delta": pre["delta"],
    }
    in_maps = [{"x": xc[c], **shared} for c in range(N_CORES)]

    import os

    tmpdir = os.environ.get("KERNEL_TRACE_DIR") or None
    if tmpdir:
        os.makedirs(tmpdir, exist_ok=True)
    res = run_bass_kernel_spmd(nc, in_maps, list(range(N_CORES)), tmpdir=tmpdir)
    global LAST_RESULT
    LAST_RESULT = res
    outs = [unpack_y(res.results[c]["y"]) for c in range(N_CORES)]
    return np.ascontiguousarray(np.concatenate(outs, axis=0))


def unpack_y(y):
    """[pairs,4,128,784] packed -> [2*pairs, 1024, 14, 14]."""
    p = y.shape[0]
    y = y.reshape(p, 4, 128, 2, 2, 196)  # (p, mp, q, m2, i, h)
    y = y.transpose(0, 4, 1, 3, 2, 5)  # (p, i, mp, m2, q, h)
    return np.ascontiguousarray(
        y.reshape(2 * p, 1024, 14, 14).astype(np.float32)
    )


# revision 20
# speedup vs baseline: 1.1418x; 1.1418x over previous
"""Trainium2 Bass kernel for the quantized ResNet Bottleneck block (v3).

Strategy
--------
Data parallel over batch: 64 images -> 8 cores x 8 images. Each core runs an
identical Bass program; weights are replicated.

All convs are integer-valued matmuls accumulated in fp32 PSUM (exact:
quantized codes are integers; code offsets pass through each conv as
per-output-channel constants folded into the next bias on the host).

v3 numeric scheme (simpler + more exact than v2):
  * x is fake-quantized ON THE HOST: codes c0 = clip(rne(254*x), -127, 127)
    shipped PLAIN (no offset) as bf16 (ints <= 127 are exact). This removes
    the on-device x-quant DVE passes entirely and the shortcut conv needs
    no column-sum correction.
  * conv1/conv2 epilogues are ONE custom DVE op each (QEPI):
    out = clip(rne(a*psum + beta), 0, 127) + 128, rounding via the f32
    magic-add (v + 1.5*2^23) trick -- exact RNE on the integer grid.
    Outputs (codes+128 in [1,255]) are exact in bf16. Pad value for conv2
    is 128; the +128 offsets fold into the next layer's bias via weight
    column sums.
  * conv3 + stride-2 shortcut accumulate into one PSUM tile (shortcut
    weights pre-scaled by css/c3s on host); epilogue is ONE custom DVE op
    (FEPI): y = min(relu(g3*psum + delta), 6), bf16 out, per-m store.

Engine placement (learned from HW traces):
  * NO elementwise work on GpSimd: Q7 tensor ops starve SBUF arbitration
    for every other engine (DVE ops degrade 350ns -> 6us while a GpSimd
    tensor_scalar runs). GpSimd only does pad memsets + tiny SWDGE DMAs.
  * DVE runs all epilogues as fused custom ops (~0.55us per [128,392]).
  * ACT only builds per-channel bias broadcast tiles once at startup.
  * Weights ride the ACT HWDGE queue, x codes + y stores the SP queue, so
    the weight stream does not delay the x stream.
  * DVE APs kept strictly 2D-contiguous where possible (3-dim views drop
    DVE to 1 elem/cycle on HW).

Schedule: pair 0 streams per (k,i) half-chunks so conv1 starts ~7us in;
software pipeline: conv2/conv3 of pair p-1 run under conv1 of pair p. A
short run of dummy matmuls on zeroed tiles warms the PE p-state ramp while
the first x DMA flies.
"""

import sys
from contextlib import ExitStack

import numpy as np

sys.path.insert(0, "/opt/trn_rl_repo")

import ml_dtypes  # noqa: E402

import concourse.bacc as bacc  # noqa: E402
import concourse.dve_ops as dve_ops  # noqa: E402
import concourse.tile as tile  # noqa: E402
from concourse import mybir  # noqa: E402
from concourse.bass_utils import run_bass_kernel_spmd  # noqa: E402
from concourse.dve_spec import (  # noqa: E402
    C0 as DC0,
    C1 as DC1,
    C2 as DC2,
    One as DOne,
    Spec,
    Src0 as DSrc0,
    Src1 as DSrc1,
    _has_src1,
    lower as dve_lower,
    maxx,
    minn,
    relu as drelu,
)
from concourse.dve_uop import DveOpSpec  # noqa: E402
from concourse.dve_table_gen import dve_ver_for  # noqa: E402
from concourse.dve_ops import DveOp  # noqa: E402

F32 = mybir.dt.float32
F16 = mybir.dt.float16
BF16 = mybir.dt.bfloat16
ALU = mybir.AluOpType
AFT = mybir.ActivationFunctionType
BF16NP = ml_dtypes.bfloat16

C_MAGIC = float(np.float32(12582912.0))  # 1.5 * 2**23

N_CORES = 8
B_LOC = 8  # images per core
PAIRS = B_LOC // 2


def _register_dve_op(name, spec, subdim=False):
    """Register a custom DVE op at runtime (table is generated per-NEFF)."""
    for o in dve_ops.OPS:
        if o.name == name:
            return o
    row = dve_ops._CUSTOM_DVE_ROW_BASE + len(dve_ops.OPS)
    assert row < 0x20
    shas = {}
    for ver in ("v3", "v4"):
        tmp = DveOpSpec(
            name=name, opcode=row, uops=dve_lower(spec, ver=ver),
            rd1_en=_has_src1(spec),
        )
        shas[ver] = tmp.sha(ver)
    op = DveOp(name, spec, subdim=subdim, uops_sha=shas)
    dve_ops.OPS.append(op)
    dve_ops._SUB_OPCODE_FOR_NAME[name] = row
    dve_ops.CUSTOM_DVE_SPECS[name] = spec
    return op


def _b(in0, in1):
    """Sim helper: in1 streams elementwise on HW; align shapes for numpy."""
    if isinstance(in1, np.ndarray) and in1.size == in0.size:
        return in1.reshape(in0.shape)
    return in1


# quantize epilogue: v = in0*s0 + in1 ; out = clip(rne(v),0,127) + 128
# rne via magic-add: u = v + C rounds to the integer grid (f32 ulp==1 in
# [2^23, 2^24)); clip in the shifted domain [C, C+127]; subtract C-128.
QEPI = _register_dve_op(
    "BNECK_QEPI_ANT",
    Spec(
        body=(minn(maxx((DSrc0 * DC0 + DSrc1) + DC1, DC1), DC1 + DC2) - DC1)
        + (DC2 + DOne),
        reference=lambda in0, in1, s0, s1, imm2: np.minimum(
            np.maximum(np.round(in0 * s0 + _b(in0, in1)), 0.0), imm2
        )
        + (imm2 + 1.0),
    ),
)


# final epilogue: out = min(relu(in0*s0 + in1), s1)
FEPI = _register_dve_op(
    "BNECK_FEPI_ANT",
    Spec(
        body=minn(drelu(DSrc0 * DC0 + DSrc1), DC1),
        reference=lambda in0, in1, s0, s1, imm2: np.minimum(
            np.maximum(in0 * s0 + _b(in0, in1), 0.0), s1
        ),
    ),
)


def _build_nc(pairs=PAIRS):
    nc = bacc.Bacc("TRN2", target_bir_lowering=False, debug=False)

    # x codes, plain int values in [-127,127], layout [pair, q, k, (i h)]
    x_d = nc.dram_tensor("x", [pairs, 128, 4, 1568], BF16, kind="ExternalInput")
    w1_d = nc.dram_tensor("w1l", [4, 128, 256], BF16, kind="ExternalInput")
    w2_d = nc.dram_tensor("w2l", [9, 2, 128, 256], BF16, kind="ExternalInput")
    w3_d = nc.dram_tensor("w3l", [2, 128, 1024], BF16, kind="ExternalInput")
    ws_d = nc.dram_tensor("wsl", [4, 128, 1024], BF16, kind="ExternalInput")
    b1_d = nc.dram_tensor("beta1", [128, 2], F32, kind="ExternalInput")
    b2_d = nc.dram_tensor("beta2", [128, 2], F32, kind="ExternalInput")
    dl_d = nc.dram_tensor("delta", [128, 8], F32, kind="ExternalInput")
    # packed [pair, m-pair, q, (m2 i h)] to keep the out-DMA 2D; host unpacks
    y_d = nc.dram_tensor("y", [pairs, 4, 128, 784], BF16, kind="ExternalOutput")

    a1, a2, g3 = _SCALES

    with tile.TileContext(nc) as tc, ExitStack() as ctx:
        wp = ctx.enter_context(tc.tile_pool(name="w", bufs=1))
        xqp = ctx.enter_context(tc.tile_pool(name="xq", bufs=3))
        p2p = ctx.enter_context(tc.tile_pool(name="p2", bufs=2))
        t3p = ctx.enter_context(tc.tile_pool(name="t3", bufs=2))
        yop = ctx.enter_context(tc.tile_pool(name="yo", bufs=6))
        pc1 = ctx.enter_context(tc.tile_pool(name="pc1", bufs=3, space="PSUM"))
        pc2 = ctx.enter_context(tc.tile_pool(name="pc2", bufs=2, space="PSUM"))
        pc3 = ctx.enter_context(tc.tile_pool(name="pc3", bufs=3, space="PSUM"))

        # ---- weights + biases ----
        # startup critical path: w1's k0 slice is the very first ACT-queue
        # DMA while pair-0's first x chunk is the first SP DMA — the two
        # sequencers issue in parallel (each issue costs ~0.6-1us of
        # sequencer time, so queue assignment matters more than transfer
        # time). Bulk weights follow on ACT, x pairs + nothing else on SP,
        # y stores on ACT, tiny biases on the GpSimd SWDGE queue.
        w1t = wp.tile([128, 4, 256], BF16, tag="w1t")
        nc.scalar.dma_start(w1t[:, 0], w1_d[0])

        beta1 = wp.tile([128, 2], F32, tag="beta1")
        nc.gpsimd.dma_start(beta1[:], b1_d[:])
        beta2 = wp.tile([128, 2], F32, tag="beta2")
        nc.gpsimd.dma_start(beta2[:], b2_d[:])
        delta = wp.tile([128, 8], F32, tag="delta")
        nc.gpsimd.dma_start(delta[:], dl_d[:])

        # zero tile: ACT broadcast source + PE warmup operands
        rz = wp.tile([128, 392], BF16, tag="rz")
        nc.vector.memset(rz[:], 0.0)
        # warm the PE p-state ramp with dummy matmuls on the zero tile
        # (~4us of continuous PE work; the clock needs >3us to reach max)
        pz = pc1.tile([128, 392], F32, tag="ps1", name="pz")
        for _ in range(12):
            nc.tensor.matmul(pz[:], rz[:, :128], rz[:], start=True, stop=True)
        # preload the ACT function table while the first DMAs fly
        warm_act = wp.tile([128, 1], F32, tag="warm_act")
        nc.scalar.activation(warm_act[:], rz[:, :1], AFT.Relu, bias=0.0, scale=1.0)

        st = {}  # per-pair tiles: xq, p2, t3

        def emit_xload(p, fine=False):
            """x codes for a pair land straight in matmul layout. Pair 0
            streams per (k,i) half-chunk so conv1 can start early."""
            xq = xqp.tile([128, 4, 1568], BF16, tag="xq", name=f"xq_{p}")
            st[p] = {"xq": xq}
            if fine == 2:
                # pair 0: k0-i0 alone (gates the first real matmul), then
                # i0 k1-3, then all of i1 — few issues, early first chunk
                nc.sync.dma_start(xq[:, 0, 0:784], x_d[p][:, 0, 0:784])
                nc.sync.dma_start(xq[:, 1:4, 0:784], x_d[p][:, 1:4, 0:784])
                nc.sync.dma_start(
                    xq[:, :, 784:1568], x_d[p][:, :, 784:1568]
                )
            elif fine:
                # pair 1: one DMA per image half
                for i in (0, 1):
                    nc.sync.dma_start(
                        xq[:, :, i * 784 : i * 784 + 784],
                        x_d[p][:, :, i * 784 : i * 784 + 784],
                    )
            else:
                nc.sync.dma_start(xq[:], x_d[p])

        def emit_conv1(p):
            xq = st[p]["xq"]
            p2 = []
            for m in range(2):
                # pad value 128 == quantized zero in the +128-shifted domain.
                # Only the cells conv2 actually reads need padding: row 0
                # and column 1 of each 29x32 image plane.
                t = p2p.tile([128, 1856], BF16, tag=f"p2_{m}")
                pv = t.rearrange("q (i r c) -> q i r c", i=2, r=29, c=32)
                nc.gpsimd.memset(pv[:, :, 0, :], 128.0)
                nc.gpsimd.memset(pv[:, :, 1:29, 1], 128.0)
                p2.append(t)
            for i in (0, 1):
                ps = {}
                # k-outer across the four (m, hf) quads; three PSUM tiles
                # from pc1, the fourth borrows a pc3 bank
                quads = [(0, 0), (0, 1), (1, 0), (1, 1)]
                for k in range(4):
                    for m, hf in quads:
                        if k == 0:
                            pool = pc3 if (m, hf) == (1, 1) else pc1
                            ps[(m, hf)] = pool.tile(
                                [128, 392], F32,
                                name=f"ps1_{m}_{hf}",
                                tag="ps3" if (m, hf) == (1, 1) else "ps1",
                            )
                        nc.tensor.matmul(
                            ps[(m, hf)][:],
                            w1t[:, k, m * 128 : (m + 1) * 128],
                            xq[:, k, i * 784 + hf * 392 : i * 784 + hf * 392 + 392],
                            start=(k == 0),
                            stop=(k == 3),
                            skip_group_check=True,
                        )
                for m in (0, 1):
                    pv = p2[m].rearrange("q (i r c) -> q i r c", i=2, r=29, c=32)
                    for hf in (0, 1):
                        # fused epilogue: codes+128 via exact magic-add RNE
                        nc.vector._custom_dve(
                            QEPI,
                            out=pv[:, i, 1 + 14 * hf : 15 + 14 * hf, 2:30],
                            in0=ps[(m, hf)][:],
                            in1=bc1[m][:],
                            s0=a1, s1=C_MAGIC, imm2=127.0,
                        )
            st[p]["p2"] = p2

        def emit_conv2(p):
            p2 = st[p]["p2"]
            t3 = []
            for m in range(2):
                ps2 = pc2.tile([128, 392], F32)
                first = True
                for k in range(2):
                    pv = p2[k].rearrange("q (i r c) -> q i r c", i=2, r=29, c=32)
                    for tp in range(9):
                        ky, kx = divmod(tp, 3)
                        nc.tensor.matmul(
                            ps2[:],
                            w2t[:, tp, k, m * 128 : (m + 1) * 128],
                            pv[:, :, ky : min(ky + 28, 29) : 2, 1 + kx : 29 + kx : 2],
                            start=first,
                            stop=(k == 1 and tp == 8),
                        )
                        first = False
                t3m = t3p.tile([128, 392], BF16, tag=f"t3_{m}")
                nc.vector._custom_dve(
                    QEPI, out=t3m[:], in0=ps2[:], in1=bc2[m][:],
                    s0=a2, s1=C_MAGIC, imm2=127.0,
                )
                t3.append(t3m)
            st[p]["t3"] = t3

        def emit_conv3(p):
            xq, t3 = st[p]["xq"], st[p]["t3"]
            for m in range(8):
                ps3 = pc3.tile([128, 392], F32)
                # shortcut first: only needs xq, giving t3's epilogue time
                for k in range(4):
                    xv = xq[:, k].rearrange("q (i r c) -> q i r c", r=28, c=28)
                    nc.tensor.matmul(
                        ps3[:],
                        wst[:, k, m * 128 : (m + 1) * 128],
                        xv[:, :, 0:28:2, 0:28:2],
                        start=(k == 0),
                        stop=False,
                        skip_group_check=True,
                    )
                for k in range(2):
                    nc.tensor.matmul(
                        ps3[:],
                        w3t[:, k, m * 128 : (m + 1) * 128],
                        t3[k][:],
                        start=False,
                        stop=(k == 1),
                        skip_group_check=True,
                    )
                # single fused DVE epilogue + per-m store keeps the chain
                # after each m's last matmul short and GpSimd-free
                yo = yop.tile([128, 392], BF16, tag="yot", name="yot")
                nc.vector._custom_dve(
                    FEPI, out=yo[:], in0=ps3[:], in1=dlf[m][:],
                    s0=g3, s1=6.0,
                )
                nc.scalar.dma_start(
                    y_d[p, m // 2][:, (m % 2) * 392 : (m % 2) * 392 + 392],
                    yo[:],
                )
            del st[p]

        # w1 k1..k3 on the ACT HWDGE queue (k0 already inbound on SP)
        nc.scalar.dma_start(
            w1t[:, 1:4], w1_d[1:4].rearrange("k q n -> q k n")
        )

        # pair 0+1 x loads first on the SP queue
        emit_xload(0, fine=2)
        emit_xload(1, fine=True)

        # conv1's bias broadcasts first (QEPI needs them ~8us in), then the
        # bulk weights on the ACT HWDGE queue (parallel to the x stream),
        # then the later-needed broadcasts. Identity(0*rz + bias_j).
        bc1, bc2, dlf = [], [], []
        for m in range(2):
            t = wp.tile([128, 392], F32, tag=f"bc1_{m}", name=f"bc1_{m}")
            nc.scalar.activation(
                t[:], rz[:], AFT.Identity, bias=beta1[:, m : m + 1], scale=0.0
            )
            bc1.append(t)
        # hold the bulk-weight DMAs back until pair-0's x is fully in:
        # their transfers otherwise steal DMA-engine bandwidth from the
        # startup-critical x chunks. The probe read blocks the in-order
        # ACT queue until the last pair-0 chunk lands.
        probe = wp.tile([128, 1], F32, tag="probe")
        nc.scalar.activation(
            probe[:], st[0]["xq"][:, 3, 1567:1568], AFT.Identity,
            bias=0.0, scale=1.0,
        )
        w2t = wp.tile([128, 9, 2, 256], BF16, tag="w2t")
        nc.scalar.dma_start(w2t[:], w2_d.rearrange("t k q n -> q t k n"))
        wst = wp.tile([128, 4, 1024], BF16, tag="wst")
        nc.scalar.dma_start(wst[:], ws_d.rearrange("k q n -> q k n"))
        w3t = wp.tile([128, 2, 1024], BF16, tag="w3t")
        nc.scalar.dma_start(w3t[:], w3_d.rearrange("k q n -> q k n"))
        for m in range(2):
            t = wp.tile([128, 392], F32, tag=f"bc2_{m}", name=f"bc2_{m}")
            nc.scalar.activation(
                t[:], rz[:], AFT.Identity, bias=beta2[:, m : m + 1], scale=0.0
            )
            bc2.append(t)
        for j in range(8):
            t = wp.tile([128, 392], F32, tag=f"dlf{j}", name=f"dlf{j}")
            nc.scalar.activation(
                t[:], rz[:], AFT.Identity, bias=delta[:, j : j + 1], scale=0.0
            )
            dlf.append(t)

        emit_conv1(0)

        # software pipeline: conv2/conv3 of pair p-1 run under conv1 of pair p
        for p in range(1, pairs):
            emit_conv1(p)
            emit_conv2(p - 1)
            if p + 1 < pairs:
                emit_xload(p + 1)
            emit_conv3(p - 1)
        emit_conv2(pairs - 1)
        emit_conv3(pairs - 1)
    return nc


_SCALES = (1.0, 1.0, 1.0)


def _prep(w1, b1, w2, b2, w3, b3, wsw, bs):
    """Host-side weight quantization + constant folding (all tiny tensors)."""
    f32 = np.float32

    def qw(w):
        s = f32(np.max(np.abs(w)))
        wq = np.round(np.clip(w / s, f32(-1.0), f32(1.0)) * f32(127.0)).astype(
            np.float32
        )
        return wq, s

    def qb(b):
        return np.round(b * f32(127.0)).astype(np.float32)

    w1q, c1s = qw(w1)  # [256,512,1,1]
    w2q, c2s = qw(w2)  # [256,256,3,3]
    w3q, c3s = qw(w3)  # [1024,256,1,1]
    wsq, css = qw(wsw)  # [1024,512,1,1]
    B1, B2, B3, Bs = qb(b1), qb(b2), qb(b3), qb(bs)

    a1 = f32(2.0) * c1s / f32(127.0)
    a2 = f32(2.0) * c2s / f32(127.0)
    g3 = c3s / f32(2.0 * 16129.0)
    rho = css / c3s

    # lhsT layouts
    w1l = np.ascontiguousarray(
        w1q[:, :, 0, 0].T.reshape(4, 128, 256).astype(BF16NP)
    )
    # w2 taps: [ky,kx] -> lhsT [cin, cout] per tap
    w2l = np.ascontiguousarray(
        w2q.transpose(2, 3, 1, 0).reshape(9, 2, 128, 256).astype(BF16NP)
    )
    w3l = np.ascontiguousarray(
        w3q[:, :, 0, 0].T.reshape(2, 128, 1024).astype(BF16NP)
    )
    ws_sc = (rho * wsq[:, :, 0, 0]).astype(BF16NP)  # [1024,512] scaled bf16
    wsl = np.ascontiguousarray(ws_sc.T.reshape(4, 128, 1024))

    # column sums for the +128 code-offset corrections (fp64 exact)
    K2 = w2q.astype(np.float64).sum(axis=(1, 2, 3))  # [256]
    K3 = w3q[:, :, 0, 0].astype(np.float64).sum(axis=1)  # [1024]

    # x codes ship plain (no offset); conv1/conv2 outputs carry +128
    beta1 = (f32(4.0) * B1).astype(np.float32)
    beta2 = (f32(4.0) * B2 - a2 * f32(128.0) * K2.astype(np.float32)).astype(
        np.float32
    )
    delta0 = B3 * c3s / (f32(127.0) * c2s) + Bs / f32(127.0)
    delta = (
        delta0 - (g3.astype(np.float64) * (128.0 * K3)).astype(np.float32)
    ).astype(np.float32)

    beta1 = np.ascontiguousarray(beta1.reshape(2, 128).T)  # [128,2]
    beta2 = np.ascontiguousarray(beta2.reshape(2, 128).T)
    delta = np.ascontiguousarray(delta.reshape(8, 128).T)  # [128,8]

    return dict(
        w1l=w1l, w2l=w2l, w3l=w3l, wsl=wsl,
        beta1=beta1, beta2=beta2, delta=delta,
        a1=float(a1), a2=float(a2), g3=float(g3),
    )


def _quant_x(x):
    """Host fake-quant of x: codes = clip(rne(x/0.5 clipped * 127)) as bf16,
    laid out [core, pair, k, q, (i h)]."""
    f32 = np.float32
    c = np.round(
        np.clip(x.astype(np.float32) / f32(0.5), f32(-1.0), f32(1.0)) * f32(127.0)
    )
    # [64, 512, 28, 28] -> [cores, pairs, 2(i), 4(k), 128(q), 784(h)]
    c = c.reshape(N_CORES, PAIRS, 2, 4, 128, 784)
    # -> [cores, pairs, q, k, i, h]
    c = c.transpose(0, 1, 4, 3, 2, 5)
    return np.ascontiguousarray(
        c.reshape(N_CORES, PAIRS, 128, 4, 1568).astype(BF16NP)
    )


def kernel(x, w1, b1, w2, b2, w3, b3, ws, bs):
    global _SCALES
    xc = _quant_x(np.asarray(x, np.float32))
    pre = _prep(
        np.asarray(w1, np.float32), np.asarray(b1, np.float32),
        np.asarray(w2, np.float32), np.asarray(b2, np.float32),
        np.asarray(w3, np.float32), np.asarray(b3, np.float32),
        np.asarray(ws, np.float32), np.asarray(bs, np.float32),
    )
    _SCALES = (pre["a1"], pre["a2"], pre["g3"])
    nc = _build_nc()
    nc.compile()

    shared = {
        "w1l": pre["w1l"], "w2l": pre["w2l"], "w3l": pre["w3l"],
        "wsl": pre["wsl"], "beta1": pre["beta1"], "beta2": pre["beta2"],
        "delta": pre["delta"],
    }
    in_maps = [{"x": xc[c], **shared} for c in range(N_CORES)]

    import os

    tmpdir = os.environ.get("KERNEL_TRACE_DIR") or None
    if tmpdir:
        os.makedirs(tmpdir, exist_ok=True)
    res = run_bass_kernel_spmd(nc, in_maps, list(range(N_CORES)), tmpdir=tmpdir)
    global LAST_RESULT
    LAST_RESULT = res
    outs = [unpack_y(res.results[c]["y"]) for c in range(N_CORES)]
    return np.ascontiguousarray(np.concatenate(outs, axis=0))


def unpack_y(y):
    """[pairs,4,128,784] packed -> [2*pairs, 1024, 14, 14]."""
    p = y.shape[0]
    y = y.reshape(p, 4, 128, 2, 2, 196)  # (p, mp, q, m2, i, h)
    y = y.transpose(0, 4, 1, 3, 2, 5)  # (p, i, mp, m2, q, h)
    return np.ascontiguousarray(
        y.reshape(2 * p, 1024, 14, 14).astype(np.float32)
    )
